# revision 23
# baseline (speedup 1.0000x reference)
"""Trainium2 Bass kernel for channel-wise ("transposed") attention.

Reference computation (per batch b, X = x_in[b] reshaped [N=16384, C=256]):
    Q = X Wq ; K = X Wk ; V = X Wv            (columns l2-normalized over tokens for Q,K)
    attn[h,i,j] = softmax_j( qhat_i . khat_j * rescale[h] )   (32x32 per head)
    out = (A_bd @ V^T)^T Wp + bp

Algebraic reduction used here (validated vs reference, rel err ~3e-6):
    S    = X^T X                      [256,256]   (only pass-1 reduction needed)
    P1   = S Wq ; P2 = S Wk
    G    = Wk^T P1                    (raw cross-gram K^T Q)
    nq2  = diag(Wq^T P1) ; nk2 = diag(Wk^T P2)
    L    = G * rk[i] * (rq*rescale_expanded)[j] ;  A = blockdiag-softmax_j(exp(L))
    Wbig = Wv @ (A_bd^T Wp)           [256,256]
    out  = X @ Wbig + bp

So the kernel is two streaming passes over X (16.8 MB in / 16.8 MB out per
core) plus tiny 256x256 matmul chains in between.  Each of the 8 cores
processes one batch (data parallel, no collectives).
"""

import sys

if "/opt/trn_rl_repo" not in sys.path:
    sys.path.insert(0, "/opt/trn_rl_repo")

from contextlib import ExitStack

import numpy as np

import concourse.bass as bass
import concourse.tile as tile
from concourse import bacc, mybir
from concourse import bass_utils
from concourse.bass import ds, ts
from concourse.bass_interp import get_hw_module
from concourse.masks import make_identity

F32 = mybir.dt.float32
F32R = mybir.dt.float32r    # PE fast-fp32 (TF32-like, ~1.5e-4 rel); 4x faster N>=256
ALU = mybir.AluOpType
ACTF = mybir.ActivationFunctionType
PSUM = bass.MemorySpace.PSUM

N_CORES = 8
B, H, W, C = 8, 128, 128, 256
HEADS, DH = 8, 32
N = H * W            # 16384 tokens per batch
P = 128              # partitions / token tile
NT = N // P          # 128 token tiles
DMA_TILES = 8        # token tiles per DMA (1 MiB chunks)
NCHUNK = C // P      # 2 channel chunks


def _build_kernel(nc: bacc.Bacc):
    x_dram = nc.dram_tensor("x_in", [N, C], F32, kind="ExternalInput").ap()
    wq_dram = nc.dram_tensor("Wq", [C, C], F32, kind="ExternalInput").ap()
    wk_dram = nc.dram_tensor("Wk", [C, C], F32, kind="ExternalInput").ap()
    wv_dram = nc.dram_tensor("Wv", [C, C], F32, kind="ExternalInput").ap()
    resc_dram = nc.dram_tensor("rescale", [HEADS, 1, 1], F32, kind="ExternalInput").ap()
    wp_dram = nc.dram_tensor("Wp", [C, C], F32, kind="ExternalInput").ap()
    bp_dram = nc.dram_tensor("bp", [C], F32, kind="ExternalInput").ap()
    out_dram = nc.dram_tensor("out", [N, C], F32, kind="ExternalOutput").ap()

    with tile.TileContext(nc) as tc, ExitStack() as top:
        consts = top.enter_context(tc.tile_pool(name="consts", bufs=1))
        xt_pool = top.enter_context(tc.tile_pool(name="xt", bufs=1))
        s_pool = top.enter_context(tc.tile_pool(name="spsum", bufs=1, space=PSUM))

        # ------------- const tiles (instructions emitted inside pass-1 g==0) -------------
        identity_f = consts.tile([P, P], F32)
        identity = consts.tile([P, P], F32R)
        p8 = consts.tile([HEADS, C], F32)        # p8[h,c] = 1 iff c//32 == h
        p8_r = consts.tile([HEADS, C], F32R)
        bdmask = consts.tile([P, NCHUNK, C], F32)  # block-diag head mask chunks
        ones_col_f = consts.tile([P, 1], F32)
        ones_col = consts.tile([P, 1], F32R)     # [128,1] ones: column-sum matmuls
        ones_row = consts.tile([1, P], F32)      # [1,128] ones: partition broadcast
        ones_row_r = consts.tile([1, P], F32R)
        d11 = consts.tile([1, 1], F32)           # ACT table prewarm scratch

        # weight tiles (DMAs issued after the x loads to keep x at queue head)
        wqk = consts.tile([P, NCHUNK, 2 * C], F32)       # [Wq | Wk] row chunks
        wp_sb = consts.tile([P, NCHUNK, C], F32)
        wv_sb = consts.tile([P, NCHUNK, C], F32)
        wvT = consts.tile([P, NCHUNK, C], F32R)          # wvT[p,k,c] = Wv[c, 128k+p]
        wqk_r = consts.tile([P, NCHUNK, 2 * C], F32R)    # rounded copies for f32r mms
        wp_r = consts.tile([P, NCHUNK, C], F32R)
        bp_sb = consts.tile([1, C], F32)
        resc_p = consts.tile([HEADS, 1], F32)
        resc_r = consts.tile([HEADS, 1], F32R)
        bp_r = consts.tile([1, C], F32R)         # rounded bias row (K=1 matmul)
        wbig_sb = consts.tile([P, NCHUNK, C], F32R)

        xT = xt_pool.tile([P, NCHUNK, N], F32R)  # X^T (f32r-rounded), from pass 1

        s_ps0 = s_pool.tile([P, C], F32, space=PSUM)
        s_ps1 = s_pool.tile([P, C], F32, space=PSUM)
        s_ps = [s_ps0, s_ps1]

        # ---------------- pass 1: S = X^T X, and X^T via PE ----------------
        with tc.tile_pool(name="tp", bufs=6, space=PSUM) as tp_pool, tc.tile_pool(
            name="xload", bufs=4
        ) as xload:
            for g in range(NT // DMA_TILES):
                xr = xload.tile([P, DMA_TILES, C], F32R, tag="xr")
                # casting DMA: loads fp32 from HBM, rounds to f32r in-flight
                nc.gpsimd.dma_start(
                    xr[:],
                    x_dram[ds(g * DMA_TILES * P, DMA_TILES * P), :].rearrange(
                        "(a p) c -> p a c", p=P
                    ),
                )
                if g == 0:
                    # masks / identity (gpsimd) — behind chunk0's descriptor gen
                    make_identity(nc, identity_f[:])
                    nc.vector.tensor_copy(identity[:], identity_f[:])
                    nc.gpsimd.memset(p8[:], 0.0)
                    nc.gpsimd.affine_select(
                        out=p8[:].rearrange("p (b i) -> p b i", i=DH),
                        in_=p8[:].rearrange("p (b i) -> p b i", i=DH),
                        compare_op=ALU.not_equal,
                        fill=1.0,
                        base=0,
                        pattern=[[-1, HEADS], [0, DH]],
                        channel_multiplier=1,
                    )
                    nc.vector.tensor_copy(p8_r[:], p8[:])
                    nc.gpsimd.memset(bdmask[:], 0.0)
                    for r in range(NCHUNK):
                        for a2 in range(P // DH):
                            nc.gpsimd.memset(
                                bdmask[ts(a2, DH), r, ds(r * P + a2 * DH, DH)], 1.0
                            )
                    nc.gpsimd.memset(ones_col_f[:], 1.0)
                    nc.vector.tensor_copy(ones_col[:], ones_col_f[:])
                    nc.gpsimd.memset(ones_row[:], 1.0)
                    nc.vector.tensor_copy(ones_row_r[:], ones_row[:])
                    # prewarm ACT sqrt table set (off critical path)
                    nc.scalar.activation(d11[:], ones_row[:, 0:1], ACTF.Sqrt)
                    # weight/bias loads + prep: issued behind the first x chunk
                    for k in range(NCHUNK):
                        nc.sync.dma_start(wqk[:, k, 0:C], wq_dram[ts(k, P), :])
                        nc.sync.dma_start(wqk[:, k, C : 2 * C], wk_dram[ts(k, P), :])
                        nc.sync.dma_start(wp_sb[:, k, :], wp_dram[ts(k, P), :])
                        nc.sync.dma_start(wv_sb[:, k, :], wv_dram[ts(k, P), :])
                    nc.sync.dma_start(bp_sb[:], bp_dram.rearrange("(a c) -> a c", a=1))
                    nc.sync.dma_start(resc_p[:], resc_dram.rearrange("h a b -> h (a b)"))
                    for k in range(NCHUNK):
                        nc.vector.tensor_copy(wqk_r[:, k, :], wqk[:, k, :])
                        nc.vector.tensor_copy(wp_r[:, k, :], wp_sb[:, k, :])
                    nc.vector.tensor_copy(bp_r[:], bp_sb[:])
                    nc.vector.tensor_copy(resc_r[:], resc_p[:])
                    for k in range(NCHUNK):
                        for m in range(NCHUNK):
                            tpv = tp_pool.tile([P, P], F32, space=PSUM, tag="tp")
                            nc.tensor.transpose(
                                tpv[:].bitcast(F32), wv_sb[:, m, ts(k, P)], identity_f[:]
                            )
                            nc.vector.tensor_copy(wvT[:, k, ts(m, P)], tpv[:].bitcast(F32))
                for a in range(DMA_TILES):
                    t = g * DMA_TILES + a
                    x_t = xr[:, a, :]
                    first, last = t == 0, t == NT - 1
                    for k in range(NCHUNK):
                        nc.tensor.matmul(
                            s_ps[k][:],
                            x_t[:, ts(k, P)],
                            x_t[:],
                            start=first,
                            stop=last,
                        )
                        tp = tp_pool.tile([P, P], F32R, space=PSUM, tag="tp")
                        nc.tensor.transpose(tp[:], x_t[:, ts(k, P)], identity[:])
                        if k == 0:
                            nc.vector.tensor_copy(xT[:, k, ts(t, P)], tp[:])
                        else:
                            nc.scalar.copy(xT[:, k, ts(t, P)], tp[:])

        # ---------------- phase B: 256x256 attention math ----------------
        with tc.tile_pool(name="bwork", bufs=1, space=PSUM) as bwork, tc.tile_pool(
            name="bsmall", bufs=2, space=PSUM
        ) as bsmall, tc.tile_pool(name="bsb", bufs=1) as bsb:
            s_sb = bsb.tile([P, NCHUNK, C], F32R)
            for k in range(NCHUNK):
                nc.vector.tensor_copy(s_sb[:, k, :], s_ps[k][:])

            # P12 = S @ [Wq | Wk]   (uses S symmetric: lhsT = S chunks)
            p12_ps = bwork.tile([P, NCHUNK, 2 * C], F32, space=PSUM, tag="bw")
            for m in range(NCHUNK):
                for k in range(NCHUNK):
                    nc.tensor.matmul(
                        p12_ps[:, m, :],
                        s_sb[:, k, ts(m, P)],
                        wqk_r[:, k, :],
                        start=(k == 0),
                        stop=(k == 1),
                    )
            p12_sb = bsb.tile([P, NCHUNK, 2 * C], F32R)
            for m in range(NCHUNK):
                nc.vector.tensor_copy(p12_sb[:, m, :], p12_ps[:, m, :])

            # [G | Kgram] = Wk^T @ [P1 | P2]
            gk_ps = bwork.tile([P, NCHUNK, 2 * C], F32, space=PSUM, tag="bw")
            for m in range(NCHUNK):
                for k in range(NCHUNK):
                    nc.tensor.matmul(
                        gk_ps[:, m, :],
                        wqk_r[:, k, ds(C + m * P, P)],
                        p12_sb[:, k, :],
                        start=(k == 0),
                        stop=(k == 1),
                    )

            # nq2[j] = sum_c Wq[c,j] P1[c,j]  -> [1, 256] via ones-matmul
            qp_sb = bsb.tile([P, NCHUNK, C], F32R)
            for k in range(NCHUNK):
                nc.vector.tensor_mul(
                    qp_sb[:, k, :],
                    wqk_r[:, k, 0:C].bitcast(F32),
                    p12_sb[:, k, 0:C].bitcast(F32),
                )
            nq2_ps = bsmall.tile([1, C], F32, space=PSUM, tag="bs")
            for k in range(NCHUNK):
                nc.tensor.matmul(
                    nq2_ps[:], ones_col[:], qp_sb[:, k, :], start=(k == 0), stop=(k == 1)
                )

            # nk2 rows: diag of Kgram chunk m  -> per-partition [128,1]
            nk2 = bsb.tile([P, NCHUNK], F32)
            scrap = bsb.tile([P, P], F32)
            for m in range(NCHUNK):
                nc.vector.scalar_tensor_tensor(
                    out=scrap[:],
                    in0=gk_ps[:, m, ds(C + m * P, P)],
                    scalar=1.0,
                    in1=identity_f[:],
                    op0=ALU.mult,
                    op1=ALU.mult,
                    accum_out=nk2[:, m : m + 1],
                )
            nk = bsb.tile([P, NCHUNK], F32)
            nc.scalar.activation(nk[:], nk2[:], ACTF.Sqrt)
            rk = bsb.tile([P, NCHUNK], F32)
            nc.vector.reciprocal(rk[:], nk[:])

            # column scale: rq[j] * rescale[head(j)]
            nq = bsb.tile([1, C], F32)
            nc.scalar.activation(nq[:], nq2_ps[:], ACTF.Sqrt)
            rq = bsb.tile([1, C], F32)
            nc.vector.reciprocal(rq[:], nq[:])
            # dummy exp: pulls the Exp table load off the critical path
            nc.scalar.activation(d11[:], ones_row[:, 0:1], ACTF.Exp)
            rexp_ps = bsmall.tile([1, C], F32, space=PSUM, tag="bs")
            nc.tensor.matmul(rexp_ps[:], resc_r[:], p8_r[:])
            colscale = bsb.tile([1, C], F32R)
            nc.vector.tensor_mul(colscale[:], rq[:], rexp_ps[:])
            csbc_ps = bsmall.tile([P, C], F32, space=PSUM, tag="bs")
            nc.tensor.matmul(csbc_ps[:], ones_row_r[:], colscale[:])
            csbc_sb = bsb.tile([P, C], F32)
            nc.vector.tensor_copy(csbc_sb[:], csbc_ps[:])

            # logits -> exp -> masked softmax -> A (block-diagonal, zero elsewhere)
            sc_sb = bsb.tile([P, NCHUNK, C], F32)
            e_sb = bsb.tile([P, NCHUNK, C], F32)
            em_sb = bsb.tile([P, NCHUNK, C], F32)
            den = bsb.tile([P, NCHUNK], F32)
            for m in range(NCHUNK):
                nc.vector.scalar_tensor_tensor(
                    out=sc_sb[:, m, :],
                    in0=gk_ps[:, m, 0:C],
                    scalar=rk[:, m : m + 1],
                    in1=csbc_sb[:],
                    op0=ALU.mult,
                    op1=ALU.mult,
                )
                nc.scalar.activation(e_sb[:, m, :], sc_sb[:, m, :], ACTF.Exp)
                nc.vector.scalar_tensor_tensor(
                    out=em_sb[:, m, :],
                    in0=e_sb[:, m, :],
                    scalar=1.0,
                    in1=bdmask[:, m, :],
                    op0=ALU.mult,
                    op1=ALU.mult,
                    accum_out=den[:, m : m + 1],
                )
            rden = bsb.tile([P, NCHUNK], F32)
            nc.vector.reciprocal(rden[:], den[:])
            a_sb = bsb.tile([P, NCHUNK, C], F32R)
            for m in range(NCHUNK):
                nc.vector.tensor_scalar_mul(
                    a_sb[:, m, :], em_sb[:, m, :], rden[:, m : m + 1]
                )

            # T1 = A_bd^T @ Wp  (lhsT = A_bd chunks directly)
            t1_ps = bwork.tile([P, NCHUNK, C], F32, space=PSUM, tag="bw")
            for m in range(NCHUNK):
                for k in range(NCHUNK):
                    nc.tensor.matmul(
                        t1_ps[:, m, :],
                        a_sb[:, k, ts(m, P)],
                        wp_r[:, k, :],
                        start=(k == 0),
                        stop=(k == 1),
                    )
            t1_sb = bsb.tile([P, NCHUNK, C], F32R)
            for m in range(NCHUNK):
                nc.vector.tensor_copy(t1_sb[:, m, :], t1_ps[:, m, :])

            # Wbig = Wv @ T1  (lhsT = Wv^T chunks)
            wbig_ps = bwork.tile([P, NCHUNK, C], F32, space=PSUM, tag="bw")
            for m in range(NCHUNK):
                for k in range(NCHUNK):
                    nc.tensor.matmul(
                        wbig_ps[:, m, :],
                        wvT[:, k, ts(m, P)],
                        t1_sb[:, k, :],
                        start=(k == 0),
                        stop=(k == 1),
                    )
            for m in range(NCHUNK):
                nc.vector.tensor_copy(wbig_sb[:, m, :], wbig_ps[:, m, :])

        # ---------------- pass 2: out = X @ Wbig + bp ----------------
        OUT_TILES = 8
        with tc.tile_pool(name="ops", bufs=6, space=PSUM) as ops, tc.tile_pool(
            name="outb", bufs=3
        ) as outb:
            for g in range(NT // OUT_TILES):
                ob = outb.tile([P, OUT_TILES, C], F32)
                for a in range(OUT_TILES):
                    t = g * OUT_TILES + a
                    o_ps = ops.tile([P, C], F32, space=PSUM, tag="o")
                    for k in range(NCHUNK):
                        nc.tensor.matmul(
                            o_ps[:],
                            xT[:, k, ts(t, P)],
                            wbig_sb[:, k, :],
                            start=(k == 0),
                            stop=False,
                        )
                    nc.tensor.matmul(
                        o_ps[:], ones_row_r[:], bp_r[:], start=False, stop=True
                    )
                    if a % 2 == 0:
                        nc.vector.tensor_copy(ob[:, a, :], o_ps[:])
                    else:
                        nc.scalar.copy(ob[:, a, :], o_ps[:])
                if g == NT // OUT_TILES - 1:
                    half = OUT_TILES // 2
                    for h2 in range(2):
                        nc.sync.dma_start(
                            out_dram[
                                ds((g * OUT_TILES + h2 * half) * P, half * P), :
                            ].rearrange("(a p) c -> p a c", p=P),
                            ob[:, ts(h2, half), :],
                        )
                else:
                    nc.sync.dma_start(
                        out_dram[ds(g * OUT_TILES * P, OUT_TILES * P), :].rearrange(
                            "(a p) c -> p a c", p=P
                        ),
                        ob[:],
                    )

    return nc


_NC_CACHE = None


def _get_nc():
    global _NC_CACHE
    if _NC_CACHE is None:
        nc = bacc.Bacc(
            "TRN2",
            target_bir_lowering=False,
            debug=False,
            enable_asserts=False,
            num_devices=N_CORES,
        )
        _build_kernel(nc)
        nc.compile()
        nc.m = get_hw_module(nc.m)
        _NC_CACHE = nc
    return _NC_CACHE


def _make_in_maps(x_in, Wq, Wk, Wv, rescale, Wp, bp):
    x_in = np.ascontiguousarray(np.asarray(x_in, dtype=np.float32))
    maps = []
    for core in range(N_CORES):
        maps.append(
            {
                "x_in": x_in[core].reshape(N, C),
                "Wq": np.asarray(Wq, np.float32),
                "Wk": np.asarray(Wk, np.float32),
                "Wv": np.asarray(Wv, np.float32),
                "rescale": np.asarray(rescale, np.float32),
                "Wp": np.asarray(Wp, np.float32),
                "bp": np.asarray(bp, np.float32),
            }
        )
    return maps


def run_on_hw(inputs: dict, trace: bool = False, tmpdir: str | None = None):
    """Returns (full_output [8,128,128,256] f32, BassKernelResults)."""
    nc = _get_nc()
    in_maps = _make_in_maps(**inputs)
    res = bass_utils.run_bass_kernel_spmd(
        nc, in_maps, core_ids=list(range(N_CORES)), trace=trace, tmpdir=tmpdir
    )
    out = np.stack([res.results[c]["out"].reshape(H, W, C) for c in range(N_CORES)])
    return out.astype(np.float32), res


def kernel(x_in, Wq, Wk, Wv, rescale, Wp, bp) -> np.ndarray:
    out, _ = run_on_hw(
        dict(x_in=x_in, Wq=Wq, Wk=Wk, Wv=Wv, rescale=rescale, Wp=Wp, bp=bp)
    )
    return out


# revision 24
# speedup vs baseline: 1.0254x; 1.0254x over previous
"""Trainium2 Bass kernel for channel-wise ("transposed") attention.

Reference computation (per batch b, X = x_in[b] reshaped [N=16384, C=256]):
    Q = X Wq ; K = X Wk ; V = X Wv            (columns l2-normalized over tokens for Q,K)
    attn[h,i,j] = softmax_j( qhat_i . khat_j * rescale[h] )   (32x32 per head)
    out = (A_bd @ V^T)^T Wp + bp

Algebraic reduction used here (validated vs reference, rel err ~3e-6):
    S    = X^T X                      [256,256]   (only pass-1 reduction needed)
    P1   = S Wq ; P2 = S Wk
    G    = Wk^T P1                    (raw cross-gram K^T Q)
    nq2  = diag(Wq^T P1) ; nk2 = diag(Wk^T P2)
    L    = G * rk[i] * (rq*rescale_expanded)[j] ;  A = blockdiag-softmax_j(exp(L))
    Wbig = Wv @ (A_bd^T Wp)           [256,256]
    out  = X @ Wbig + bp

So the kernel is two streaming passes over X (16.8 MB in / 16.8 MB out per
core) plus tiny 256x256 matmul chains in between.  Each of the 8 cores
processes one batch (data parallel, no collectives).
"""

import sys

if "/opt/trn_rl_repo" not in sys.path:
    sys.path.insert(0, "/opt/trn_rl_repo")

from contextlib import ExitStack

import numpy as np

import concourse.bass as bass
import concourse.tile as tile
from concourse import bacc, mybir
from concourse import bass_utils
from concourse.bass import ds, ts
from concourse.bass_interp import get_hw_module
from concourse.masks import make_identity

F32 = mybir.dt.float32
F32R = mybir.dt.float32r    # PE fast-fp32 (TF32-like, ~1.5e-4 rel); 4x faster N>=256
ALU = mybir.AluOpType
ACTF = mybir.ActivationFunctionType
PSUM = bass.MemorySpace.PSUM

N_CORES = 8
B, H, W, C = 8, 128, 128, 256
HEADS, DH = 8, 32
N = H * W            # 16384 tokens per batch
P = 128              # partitions / token tile
NT = N // P          # 128 token tiles
DMA_TILES = 8        # token tiles per DMA (1 MiB chunks)
NCHUNK = C // P      # 2 channel chunks


def _build_kernel(nc: bacc.Bacc):
    x_dram = nc.dram_tensor("x_in", [N, C], F32, kind="ExternalInput").ap()
    wq_dram = nc.dram_tensor("Wq", [C, C], F32, kind="ExternalInput").ap()
    wk_dram = nc.dram_tensor("Wk", [C, C], F32, kind="ExternalInput").ap()
    wv_dram = nc.dram_tensor("Wv", [C, C], F32, kind="ExternalInput").ap()
    resc_dram = nc.dram_tensor("rescale", [HEADS, 1, 1], F32, kind="ExternalInput").ap()
    wp_dram = nc.dram_tensor("Wp", [C, C], F32, kind="ExternalInput").ap()
    bp_dram = nc.dram_tensor("bp", [C], F32, kind="ExternalInput").ap()
    out_dram = nc.dram_tensor("out", [N, C], F32, kind="ExternalOutput").ap()

    with tile.TileContext(nc) as tc, ExitStack() as top:
        consts = top.enter_context(tc.tile_pool(name="consts", bufs=1))
        xt_pool = top.enter_context(tc.tile_pool(name="xt", bufs=1))
        s_pool = top.enter_context(tc.tile_pool(name="spsum", bufs=1, space=PSUM))

        # ------------- const tiles (instructions emitted inside pass-1 g==0) -------------
        identity_f = consts.tile([P, P], F32)
        identity = consts.tile([P, P], F32R)
        p8 = consts.tile([HEADS, C], F32)        # p8[h,c] = 1 iff c//32 == h
        p8_r = consts.tile([HEADS, C], F32R)
        bdmask = consts.tile([P, NCHUNK, C], F32)  # block-diag head mask chunks
        ones_col_f = consts.tile([P, 1], F32)
        ones_col = consts.tile([P, 1], F32R)     # [128,1] ones: column-sum matmuls
        ones_row = consts.tile([1, P], F32)      # [1,128] ones: partition broadcast
        ones_row_r = consts.tile([1, P], F32R)
        d11 = consts.tile([1, 1], F32)           # ACT table prewarm scratch

        # weight tiles (DMAs issued after the x loads to keep x at queue head)
        wqk = consts.tile([P, NCHUNK, 2 * C], F32)       # [Wq | Wk] row chunks
        wp_sb = consts.tile([P, NCHUNK, C], F32)
        wv_sb = consts.tile([P, NCHUNK, C], F32)
        wvT = consts.tile([P, NCHUNK, C], F32R)          # wvT[p,k,c] = Wv[c, 128k+p]
        wqk_r = consts.tile([P, NCHUNK, 2 * C], F32R)    # rounded copies for f32r mms
        wp_r = consts.tile([P, NCHUNK, C], F32R)
        bp_sb = consts.tile([1, C], F32)
        resc_p = consts.tile([HEADS, 1], F32)
        resc_r = consts.tile([HEADS, 1], F32R)
        bp_r = consts.tile([1, C], F32R)         # rounded bias row (K=1 matmul)
        wbig_sb = consts.tile([P, NCHUNK, C], F32R)

        xT = xt_pool.tile([P, NCHUNK, N], F32R)  # X^T (f32r-rounded), from pass 1

        s_ps0 = s_pool.tile([P, C], F32, space=PSUM)
        s_ps1 = s_pool.tile([P, C], F32, space=PSUM)
        s_ps = [s_ps0, s_ps1]

        # ---------------- pass 1: S = X^T X, and X^T via PE ----------------
        with tc.tile_pool(name="tp", bufs=6, space=PSUM) as tp_pool, tc.tile_pool(
            name="xload", bufs=4
        ) as xload:
            for g in range(NT // DMA_TILES):
                xr = xload.tile([P, DMA_TILES, C], F32R, tag="xr")
                # casting DMA: loads fp32 from HBM, rounds to f32r in-flight
                if g == 0:
                    # small first piece so PE starts sooner
                    for lo, n_t in ((0, 2), (2, 6)):
                        nc.gpsimd.dma_start(
                            xr[:, ds(lo, n_t), :],
                            x_dram[ds((g * DMA_TILES + lo) * P, n_t * P), :].rearrange(
                                "(a p) c -> p a c", p=P
                            ),
                        )
                else:
                    nc.gpsimd.dma_start(
                        xr[:],
                        x_dram[ds(g * DMA_TILES * P, DMA_TILES * P), :].rearrange(
                            "(a p) c -> p a c", p=P
                        ),
                    )
                if g == 0:
                    # masks / identity (gpsimd) — behind chunk0's descriptor gen
                    make_identity(nc, identity_f[:])
                    nc.vector.tensor_copy(identity[:], identity_f[:])
                    nc.gpsimd.memset(p8[:], 0.0)
                    nc.gpsimd.affine_select(
                        out=p8[:].rearrange("p (b i) -> p b i", i=DH),
                        in_=p8[:].rearrange("p (b i) -> p b i", i=DH),
                        compare_op=ALU.not_equal,
                        fill=1.0,
                        base=0,
                        pattern=[[-1, HEADS], [0, DH]],
                        channel_multiplier=1,
                    )
                    nc.vector.tensor_copy(p8_r[:], p8[:])
                    nc.gpsimd.memset(bdmask[:], 0.0)
                    for r in range(NCHUNK):
                        for a2 in range(P // DH):
                            nc.gpsimd.memset(
                                bdmask[ts(a2, DH), r, ds(r * P + a2 * DH, DH)], 1.0
                            )
                    nc.gpsimd.memset(ones_col_f[:], 1.0)
                    nc.vector.tensor_copy(ones_col[:], ones_col_f[:])
                    nc.gpsimd.memset(ones_row[:], 1.0)
                    nc.vector.tensor_copy(ones_row_r[:], ones_row[:])
                    # prewarm ACT sqrt table set (off critical path)
                    nc.scalar.activation(d11[:], ones_row[:, 0:1], ACTF.Sqrt)
                if g == 1:
                    # weight/bias loads + prep: issued behind the first x chunk
                    for k in range(NCHUNK):
                        nc.sync.dma_start(wqk[:, k, 0:C], wq_dram[ts(k, P), :])
                        nc.sync.dma_start(wqk[:, k, C : 2 * C], wk_dram[ts(k, P), :])
                        nc.sync.dma_start(wp_sb[:, k, :], wp_dram[ts(k, P), :])
                        nc.sync.dma_start(wv_sb[:, k, :], wv_dram[ts(k, P), :])
                    nc.sync.dma_start(bp_sb[:], bp_dram.rearrange("(a c) -> a c", a=1))
                    nc.sync.dma_start(resc_p[:], resc_dram.rearrange("h a b -> h (a b)"))
                    for k in range(NCHUNK):
                        nc.vector.tensor_copy(wqk_r[:, k, :], wqk[:, k, :])
                        nc.vector.tensor_copy(wp_r[:, k, :], wp_sb[:, k, :])
                    nc.vector.tensor_copy(bp_r[:], bp_sb[:])
                    nc.vector.tensor_copy(resc_r[:], resc_p[:])
                    for k in range(NCHUNK):
                        for m in range(NCHUNK):
                            tpv = tp_pool.tile([P, P], F32, space=PSUM, tag="tp")
                            nc.tensor.transpose(
                                tpv[:].bitcast(F32), wv_sb[:, m, ts(k, P)], identity_f[:]
                            )
                            nc.vector.tensor_copy(wvT[:, k, ts(m, P)], tpv[:].bitcast(F32))
                for a in range(DMA_TILES):
                    t = g * DMA_TILES + a
                    x_t = xr[:, a, :]
                    first, last = t == 0, t == NT - 1
                    for k in range(NCHUNK):
                        nc.tensor.matmul(
                            s_ps[k][:],
                            x_t[:, ts(k, P)],
                            x_t[:],
                            start=first,
                            stop=last,
                        )
                        tp = tp_pool.tile([P, P], F32R, space=PSUM, tag="tp")
                        nc.tensor.transpose(tp[:], x_t[:, ts(k, P)], identity[:])
                        if k == 0:
                            nc.vector.tensor_copy(xT[:, k, ts(t, P)], tp[:])
                        else:
                            nc.scalar.copy(xT[:, k, ts(t, P)], tp[:])

        # ---------------- phase B: 256x256 attention math ----------------
        with tc.tile_pool(name="bwork", bufs=1, space=PSUM) as bwork, tc.tile_pool(
            name="bsmall", bufs=2, space=PSUM
        ) as bsmall, tc.tile_pool(name="bsb", bufs=1) as bsb:
            s_sb = bsb.tile([P, NCHUNK, C], F32R)
            for k in range(NCHUNK):
                nc.vector.tensor_copy(s_sb[:, k, :], s_ps[k][:])

            # P12 = S @ [Wq | Wk]   (uses S symmetric: lhsT = S chunks)
            p12_ps = bwork.tile([P, NCHUNK, 2 * C], F32, space=PSUM, tag="bw")
            for m in range(NCHUNK):
                for k in range(NCHUNK):
                    nc.tensor.matmul(
                        p12_ps[:, m, :],
                        s_sb[:, k, ts(m, P)],
                        wqk_r[:, k, :],
                        start=(k == 0),
                        stop=(k == 1),
                    )
            p12_sb = bsb.tile([P, NCHUNK, 2 * C], F32R)
            for m in range(NCHUNK):
                nc.vector.tensor_copy(p12_sb[:, m, :], p12_ps[:, m, :])

            # [G | Kgram] = Wk^T @ [P1 | P2]
            gk_ps = bwork.tile([P, NCHUNK, 2 * C], F32, space=PSUM, tag="bw")
            for m in range(NCHUNK):
                for k in range(NCHUNK):
                    nc.tensor.matmul(
                        gk_ps[:, m, :],
                        wqk_r[:, k, ds(C + m * P, P)],
                        p12_sb[:, k, :],
                        start=(k == 0),
                        stop=(k == 1),
                    )

            # nq2[j] = sum_c Wq[c,j] P1[c,j]  -> [1, 256] via ones-matmul
            qp_sb = bsb.tile([P, NCHUNK, C], F32R)
            for k in range(NCHUNK):
                nc.vector.tensor_mul(
                    qp_sb[:, k, :],
                    wqk_r[:, k, 0:C].bitcast(F32),
                    p12_sb[:, k, 0:C].bitcast(F32),
                )
            nq2_ps = bsmall.tile([1, C], F32, space=PSUM, tag="bs")
            for k in range(NCHUNK):
                nc.tensor.matmul(
                    nq2_ps[:], ones_col[:], qp_sb[:, k, :], start=(k == 0), stop=(k == 1)
                )

            # nk2 rows: diag of Kgram chunk m  -> per-partition [128,1]
            nk2 = bsb.tile([P, NCHUNK], F32)
            scrap = bsb.tile([P, P], F32)
            for m in range(NCHUNK):
                nc.vector.scalar_tensor_tensor(
                    out=scrap[:],
                    in0=gk_ps[:, m, ds(C + m * P, P)],
                    scalar=1.0,
                    in1=identity_f[:],
                    op0=ALU.mult,
                    op1=ALU.mult,
                    accum_out=nk2[:, m : m + 1],
                )
            nk = bsb.tile([P, NCHUNK], F32)
            nc.scalar.activation(nk[:], nk2[:], ACTF.Sqrt)
            rk = bsb.tile([P, NCHUNK], F32)
            nc.vector.reciprocal(rk[:], nk[:])

            # column scale: rq[j] * rescale[head(j)]
            nq = bsb.tile([1, C], F32)
            nc.scalar.activation(nq[:], nq2_ps[:], ACTF.Sqrt)
            rq = bsb.tile([1, C], F32)
            nc.vector.reciprocal(rq[:], nq[:])
            # dummy exp: pulls the Exp table load off the critical path
            nc.scalar.activation(d11[:], ones_row[:, 0:1], ACTF.Exp)
            rexp_ps = bsmall.tile([1, C], F32, space=PSUM, tag="bs")
            nc.tensor.matmul(rexp_ps[:], resc_r[:], p8_r[:])
            colscale = bsb.tile([1, C], F32R)
            nc.vector.tensor_mul(colscale[:], rq[:], rexp_ps[:])
            csbc_ps = bsmall.tile([P, C], F32, space=PSUM, tag="bs")
            nc.tensor.matmul(csbc_ps[:], ones_row_r[:], colscale[:])
            csbc_sb = bsb.tile([P, C], F32)
            nc.vector.tensor_copy(csbc_sb[:], csbc_ps[:])

            # logits -> exp -> masked softmax -> A (block-diagonal, zero elsewhere)
            sc_sb = bsb.tile([P, NCHUNK, C], F32)
            e_sb = bsb.tile([P, NCHUNK, C], F32)
            em_sb = bsb.tile([P, NCHUNK, C], F32)
            den = bsb.tile([P, NCHUNK], F32)
            for m in range(NCHUNK):
                nc.vector.scalar_tensor_tensor(
                    out=sc_sb[:, m, :],
                    in0=gk_ps[:, m, 0:C],
                    scalar=rk[:, m : m + 1],
                    in1=csbc_sb[:],
                    op0=ALU.mult,
                    op1=ALU.mult,
                )
                nc.scalar.activation(e_sb[:, m, :], sc_sb[:, m, :], ACTF.Exp)
                nc.vector.scalar_tensor_tensor(
                    out=em_sb[:, m, :],
                    in0=e_sb[:, m, :],
                    scalar=1.0,
                    in1=bdmask[:, m, :],
                    op0=ALU.mult,
                    op1=ALU.mult,
                    accum_out=den[:, m : m + 1],
                )
            rden = bsb.tile([P, NCHUNK], F32)
            nc.vector.reciprocal(rden[:], den[:])
            a_sb = bsb.tile([P, NCHUNK, C], F32R)
            for m in range(NCHUNK):
                nc.vector.tensor_scalar_mul(
                    a_sb[:, m, :], em_sb[:, m, :], rden[:, m : m + 1]
                )

            # T1 = A_bd^T @ Wp  (lhsT = A_bd chunks directly)
            t1_ps = bwork.tile([P, NCHUNK, C], F32, space=PSUM, tag="bw")
            for m in range(NCHUNK):
                for k in range(NCHUNK):
                    nc.tensor.matmul(
                        t1_ps[:, m, :],
                        a_sb[:, k, ts(m, P)],
                        wp_r[:, k, :],
                        start=(k == 0),
                        stop=(k == 1),
                    )
            t1_sb = bsb.tile([P, NCHUNK, C], F32R)
            for m in range(NCHUNK):
                nc.vector.tensor_copy(t1_sb[:, m, :], t1_ps[:, m, :])

            # Wbig = Wv @ T1  (lhsT = Wv^T chunks)
            wbig_ps = bwork.tile([P, NCHUNK, C], F32, space=PSUM, tag="bw")
            for m in range(NCHUNK):
                for k in range(NCHUNK):
                    nc.tensor.matmul(
                        wbig_ps[:, m, :],
                        wvT[:, k, ts(m, P)],
                        t1_sb[:, k, :],
                        start=(k == 0),
                        stop=(k == 1),
                    )
            for m in range(NCHUNK):
                nc.vector.tensor_copy(wbig_sb[:, m, :], wbig_ps[:, m, :])

        # ---------------- pass 2: out = X @ Wbig + bp ----------------
        OUT_TILES = 8
        with tc.tile_pool(name="ops", bufs=6, space=PSUM) as ops, tc.tile_pool(
            name="outb", bufs=3
        ) as outb:
            for g in range(NT // OUT_TILES):
                ob = outb.tile([P, OUT_TILES, C], F32)
                for a in range(OUT_TILES):
                    t = g * OUT_TILES + a
                    o_ps = ops.tile([P, C], F32, space=PSUM, tag="o")
                    for k in range(NCHUNK):
                        nc.tensor.matmul(
                            o_ps[:],
                            xT[:, k, ts(t, P)],
                            wbig_sb[:, k, :],
                            start=(k == 0),
                            stop=False,
                        )
                    nc.tensor.matmul(
                        o_ps[:], ones_row_r[:], bp_r[:], start=False, stop=True
                    )
                    if a % 2 == 0:
                        nc.vector.tensor_copy(ob[:, a, :], o_ps[:])
                    else:
                        nc.scalar.copy(ob[:, a, :], o_ps[:])
                if g == NT // OUT_TILES - 1:
                    half = OUT_TILES // 2
                    for h2 in range(2):
                        nc.sync.dma_start(
                            out_dram[
                                ds((g * OUT_TILES + h2 * half) * P, half * P), :
                            ].rearrange("(a p) c -> p a c", p=P),
                            ob[:, ts(h2, half), :],
                        )
                else:
                    nc.sync.dma_start(
                        out_dram[ds(g * OUT_TILES * P, OUT_TILES * P), :].rearrange(
                            "(a p) c -> p a c", p=P
                        ),
                        ob[:],
                    )

    return nc


_NC_CACHE = None


def _get_nc():
    global _NC_CACHE
    if _NC_CACHE is None:
        nc = bacc.Bacc(
            "TRN2",
            target_bir_lowering=False,
            debug=False,
            enable_asserts=False,
            num_devices=N_CORES,
        )
        _build_kernel(nc)
        nc.compile()
        nc.m = get_hw_module(nc.m)
        _NC_CACHE = nc
    return _NC_CACHE


def _make_in_maps(x_in, Wq, Wk, Wv, rescale, Wp, bp):
    x_in = np.ascontiguousarray(np.asarray(x_in, dtype=np.float32))
    maps = []
    for core in range(N_CORES):
        maps.append(
            {
                "x_in": x_in[core].reshape(N, C),
                "Wq": np.asarray(Wq, np.float32),
                "Wk": np.asarray(Wk, np.float32),
                "Wv": np.asarray(Wv, np.float32),
                "rescale": np.asarray(rescale, np.float32),
                "Wp": np.asarray(Wp, np.float32),
                "bp": np.asarray(bp, np.float32),
            }
        )
    return maps


def run_on_hw(inputs: dict, trace: bool = False, tmpdir: str | None = None):
    """Returns (full_output [8,128,128,256] f32, BassKernelResults)."""
    nc = _get_nc()
    in_maps = _make_in_maps(**inputs)
    res = bass_utils.run_bass_kernel_spmd(
        nc, in_maps, core_ids=list(range(N_CORES)), trace=trace, tmpdir=tmpdir
    )
    out = np.stack([res.results[c]["out"].reshape(H, W, C) for c in range(N_CORES)])
    return out.astype(np.float32), res


def kernel(x_in, Wq, Wk, Wv, rescale, Wp, bp) -> np.ndarray:
    out, _ = run_on_hw(
        dict(x_in=x_in, Wq=Wq, Wk=Wk, Wv=Wv, rescale=rescale, Wp=Wp, bp=bp)
    )
    return out
